# revision 43
# baseline (speedup 1.0000x reference)
"""Trainium2 Bass kernel for 3-layer GAT + pooling readout (nn_GNN_7653631722064).

v4.5 (~450us, vs 900us v3 baseline). Key ideas over v3:
- Aggregate PRE-transform features: sum_e coef_e*(x_e @ W) =
  (sum_e coef_e*x_e) @ W, so gather x[src] rows (128B fp8 x + 16B bf16
  es, 256B elems) instead of h[src] (1280B): 5x less gather traffic,
  5x smaller AllGather tables; W applied per head at evict over
  256-dst groups, head-sum + BN fused into one psum accumulation.
- Layer 0 needs NO gather at all: host pre-gathers per-edge x tiles
  (plain input layout) in both [e,x] and [x,e] orientations; es0 per
  edge comes from a matmul on the transposed tile that accumulates
  es0[src]+ed0[dst] straight into the alpha psum region.
- dma_gather costs ~6-8ns/idx of serial gpsimd time -> idx count is
  the wall: degree-balanced within-graph node placement packs slot
  edge counts just under multiples of 128 (26% -> ~5% chunk padding),
  trailing pad idxs trimmed via num_idxs, gathers striped across 4
  SWDGE queues (num_swdge_queues=4 gives ~2.3x gather throughput).
- 32-dst slots; per 128-edge chunk ONE wide aggregation matmul
  (lhsT = gathered Mx stationary, rhs = coef-scaled one-hot S8
  [e, 8h*32d]); ED/RD (ed[dst], rden[dst] per edge) via 32-row-strip
  tile_position matmuls; den via col-tiled matmul; coefficients are
  normalized BEFORE aggregation (coef = exp(lrelu)*rden[dst]) so no
  partition-broadcast is ever needed.
- 3-stage software pipeline per slot (A: gather+alpha/exp; B lag-1:
  den+recip+RD; C lag-2: coef+S8+agg) keeps the PE from stalling on
  vector/scalar round-trips; all per-slot psum state (zT, alpha, RDp,
  den) lives in one 2KB psum bank; first 4 slots of layers 1/2 use
  tier-split gathers bounded to already-landed AllGather groups.
"""
import sys

sys.path.insert(0, "/opt/trn_rl_repo")

import numpy as np
import ml_dtypes

import concourse.bass as bass
import concourse.tile as tile
from concourse import bacc, mybir
from concourse.bass_utils import run_bass_kernel_spmd

BF16 = mybir.dt.bfloat16
FP8 = mybir.dt.float8e4
F32 = mybir.dt.float32
I16 = mybir.dt.int16
AF = mybir.ActivationFunctionType
OP = mybir.AluOpType

N, E, IN, H, C, G = 10000, 120000, 256, 8, 128, 64
NCORES = 8
TILE = 32                      # dst nodes per slot
NSLOT = 40                     # slots per core (40*32=1280 >= 1250)
PER_CORE = 1250
RPAD = NSLOT * TILE            # padded rows per core (1280)
NFULL = RPAD * NCORES          # 10240 rows in full tables
ROW = 256                      # L1/2 row bytes: 128 fp8 x + 16B es + pad
GSLOT = 8                      # slots per evict group (256 dst)
NGRP = NSLOT // GSLOT          # 5 groups
AG_GROUPS = [(0, 16), (16, 32), (32, 40)]   # slot ranges per AllGather
EPS = 1e-5
SLOPE = 0.2


def _bf(a):
    return np.asarray(a, dtype=ml_dtypes.bfloat16)


def _f8(a):
    return np.asarray(a, dtype=ml_dtypes.float8_e4m3)


def build_nc(nct, stpos, g0, cnt, trim, chunk_lim):
    nchunk = int(sum(nct))
    NCT_MAX = int(max(nct))
    slot_c0 = np.concatenate([[0], np.cumsum(nct)]).astype(int)
    CH4 = int(max(stpos)) + 1
    nc = bacc.Bacc(None, target_bir_lowering=False, debug=False,
                   num_devices=NCORES, name="gatx", num_swdge_queues=4)

    mx0_in = nc.dram_tensor("mx0", [128, nchunk, 256], FP8,
                            kind="ExternalInput")
    mxt0_in = nc.dram_tensor("mxt0", [128, nchunk, 2, 128], FP8,
                             kind="ExternalInput")
    wsdq_in = nc.dram_tensor("wsdq", [128, 2, 8], FP8, kind="ExternalInput")
    xT_in = nc.dram_tensor("xT", [128, 2, RPAD], BF16, kind="ExternalInput")
    oh_in = nc.dram_tensor("oh", [128, nchunk, TILE], FP8, kind="ExternalInput")
    st_in = nc.dram_tensor("st", [128, CH4, 128], BF16, kind="ExternalInput")
    gi_in = nc.dram_tensor("gi", [128, nchunk * 8], I16, kind="ExternalInput")
    wl_in = nc.dram_tensor("wl", [128, 4, H, 128], BF16, kind="ExternalInput")
    wsd_in = nc.dram_tensor("wsd", [128, 4, 16], BF16, kind="ExternalInput")
    bn_in = nc.dram_tensor("bn", [128, 9], F32, kind="ExternalInput")
    id_in = nc.dram_tensor("ident", [128, 128], BF16, kind="ExternalInput")
    xr_in = nc.dram_tensor("xrootT", [128, 2, 8], BF16, kind="ExternalInput")
    l0w_in = nc.dram_tensor("l0w", [128, 2, 128], BF16, kind="ExternalInput")
    lnw_in = nc.dram_tensor("lnw", [128, 2, 128], BF16, kind="ExternalInput")
    l1w_in = nc.dram_tensor("l1w", [128, 2, 1], BF16, kind="ExternalInput")
    l0b_in = nc.dram_tensor("l0b", [128, 1], F32, kind="ExternalInput")
    lnb_in = nc.dram_tensor("lnb", [128, 1], F32, kind="ExternalInput")
    l1b_in = nc.dram_tensor("l1b", [8, 1], F32, kind="ExternalInput")
    out_t = nc.dram_tensor("out", [8, 1], F32, kind="ExternalOutput")

    warm_in = nc.dram_tensor("warm_in", [8, 128], FP8, kind="Internal")
    warm_out = nc.dram_tensor("warm_out", [64, 128], FP8, kind="Internal",
                              addr_space="Shared")
    shard = [nc.dram_tensor(f"shard{l}", [RPAD, ROW], FP8, kind="Internal")
             for l in (1, 2)]
    fullx = [nc.dram_tensor(f"full{l}", [NFULL, ROW], FP8, kind="Internal",
                            addr_space="Shared")
             for l in (1, 2)]

    def full_t(l):
        return fullx[l - 1]

    with tile.TileContext(nc) as tc:
        with (
            tc.tile_pool(name="persist", bufs=1) as pp,
            tc.tile_pool(name="work", bufs=5) as wp,
            tc.tile_pool(name="mbuf", bufs=7) as mp,
            tc.tile_pool(name="small", bufs=6) as sp,
            tc.tile_pool(name="zg", bufs=2) as zgp,
            tc.tile_pool(name="pz", bufs=4, space="PSUM") as pz,
            tc.tile_pool(name="pout", bufs=2, space="PSUM") as pout,
            tc.tile_pool(name="ptr", bufs=1, space="PSUM") as ptr,
            tc.tile_pool(name="pscr", bufs=1, space="PSUM") as pscr,
        ):
            XT = pp.tile([128, 2, RPAD], BF16)
            OH = pp.tile([128, nchunk, TILE], FP8)
            ST = pp.tile([128, CH4, 128], BF16)
            GI = pp.tile([128, nchunk * 8], I16)
            WL = pp.tile([128, 4, H, 128], BF16)
            WSD = pp.tile([128, 4, 16], BF16)
            WSDQ = pp.tile([128, 2, 8], FP8)
            BN = pp.tile([128, 9], F32)
            IDT = pp.tile([128, 128], BF16)
            XR = pp.tile([128, 2, 8], BF16)
            L0W = pp.tile([128, 2, 128], BF16)
            LNW = pp.tile([128, 2, 128], BF16)
            L1W = pp.tile([128, 2, 1], BF16)
            L0B = pp.tile([128, 1], F32)
            LNB = pp.tile([128, 1], F32)
            L1B = pp.tile([8, 1], F32)
            ESD = [pp.tile([128, 10, 16], BF16, name=f"esd{i}")
                   for i in range(2)]
            a3 = pp.tile([128, PER_CORE], BF16)
            gmpb = pp.tile([128, 8], BF16)
            gapb = pp.tile([128, 8], BF16)
            for t, src_ in [(XT, xT_in), (OH, oh_in), (ST, st_in),
                            (GI, gi_in), (WL, wl_in), (WSD, wsd_in),
                            (WSDQ, wsdq_in),
                            (BN, bn_in), (IDT, id_in), (XR, xr_in),
                            (L0W, l0w_in), (LNW, lnw_in), (L1W, l1w_in),
                            (L0B, l0b_in), (LNB, lnb_in), (L1B, l1b_in)]:
                nc.sync.dma_start(t[:], src_[:])

            wtile = sp.tile([8, 128], FP8, tag="warm")
            nc.vector.memset(wtile[:], 0.0)
            nc.sync.dma_start(warm_in[:], wtile[:])
            nc.gpsimd.collective_compute(
                "AllGather", OP.bypass,
                replica_groups=[list(range(NCORES))],
                ins=[warm_in[:].opt()], outs=[warm_out[:].opt()])

            # ---- es0/ed0 for own nodes (feeds ED strips of layer 0) ----
            for g in range(10):
                pe = pscr.tile([128, 16], F32, tag="scr")
                nc.tensor.matmul(pe[:], XT[:, 0, g * 128:(g + 1) * 128],
                                 WSD[:, 0, :], start=True, stop=False)
                nc.tensor.matmul(pe[:], XT[:, 1, g * 128:(g + 1) * 128],
                                 WSD[:, 1, :], start=False, stop=True)
                nc.vector.tensor_copy(ESD[0][:, g, :], pe[:])

            def emit_ag(l, gi_):
                s0, s1 = AG_GROUPS[gi_]
                r0, r1 = s0 * TILE, s1 * TILE
                shd = shard[l - 1]
                f0 = r0 * NCORES
                nc.gpsimd.collective_compute(
                    "AllGather", OP.bypass,
                    replica_groups=[list(range(NCORES))],
                    ins=[shd[r0:r1, :].opt()],
                    outs=[full_t(l)[f0:f0 + (r1 - r0) * NCORES, :].opt()])

            # ------------------------------------------------------------
            # per-slot psum bank layout (f32 elems):
            #   [0:256]   zT accumulation [x, h*d]
            #   [256:288] alpha (L0: es0+ed accumulated; L1/2: EDp)
            #   [288:320] RDp  (rden[dst] per edge, [nch, 8])
            #   [320:328] den  (on partitions 32j..32j+32)
            ZO = H * TILE
            EO = ZO
            RO = EO + 8 * NCT_MAX
            DO = RO + 8 * NCT_MAX
            assert DO + 8 <= 512
            PIECE = 2
            CMAX = int(max(slot_c0[p + PIECE] - slot_c0[p]
                           for p in range(0, NSLOT, PIECE)))

            # scrub gather buffers once so pad positions never hold
            # uninitialized SBUF bytes (exp(garbage) -> inf -> NaN)
            for r in range(7):
                Mw = mp.tile([128, CMAX, ROW], FP8, tag="M1",
                             name=f"Mws{r}")
                nc.vector.memset(Mw[:], 0.0)

            def gather_piece(l, p, s_end):
                c0 = int(slot_c0[p])
                c1 = int(slot_c0[s_end])
                nch = c1 - c0
                if l == 0:
                    M = mp.tile([128, CMAX, 256], FP8, tag="M0",
                                name=f"M0_{p}")
                    nc.sync.dma_start(M[:, 0:nch, :], mx0_in[:, c0:c1, :])
                    MT = mp.tile([128, CMAX, 2, 128], FP8, tag="MT0",
                                 name=f"MT0_{p}")
                    nc.sync.dma_start(MT[:, 0:nch, :, :],
                                      mxt0_in[:, c0:c1, :, :])
                    return M, MT
                M = mp.tile([128, CMAX, ROW], FP8, tag="M1",
                            name=f"M1_{l}_{p}")
                if s_end - p == 1:
                    # tier-split: sub-gathers limited to landed AG groups
                    subs = []
                    cl = [int(chunk_lim[c]) for c in range(c0, c1)]
                    i = 0
                    while i < nch:
                        k2 = i
                        while k2 < nch and cl[k2] == cl[i]:
                            k2 += 1
                        subs.append((i, k2, cl[i]))
                        i = k2
                    for (i, k2, lim) in subs:
                        ni = (k2 - i) * 128
                        if k2 == nch:
                            ni = int(trim[(p, s_end)]) - i * 128
                        nc.gpsimd.dma_gather(
                            out_ap=M[:, i:k2, :], in_ap=full_t(l)[0:lim],
                            idxs_ap=GI[:, (c0 + i) * 8:(c0 + k2) * 8],
                            num_idxs=ni, num_idxs_reg=ni,
                            elem_size=ROW, elem_step=ROW,
                            queue_num=p % 4)
                    return M, None
                nc.gpsimd.dma_gather(
                    out_ap=M[:, 0:nch, :], in_ap=full_t(l)[0:NFULL],
                    idxs_ap=GI[:, c0 * 8:c1 * 8],
                    num_idxs=int(trim[(p, s_end)]),
                    num_idxs_reg=int(trim[(p, s_end)]),
                    elem_size=ROW, elem_step=ROW,
                    queue_num=(p // PIECE) % 4)
                return M, None

            def stage_a(l, s, M, MT, coff):
                """alpha (es[src]+ed[dst]) + exp chain."""
                esd_src = ESD[l % 2]
                nch = int(nct[s])
                c0 = int(slot_c0[s])
                j = s % 4
                sg = s // 4
                zt = pz.tile([128, 512], F32, tag="zt", name=f"z{l}_{s}")
                for ci in range(nch):
                    ar = zt[:, EO + 8 * ci:EO + 8 * ci + 8]
                    if l == 0:
                        # es0 = x@ws via transposed pregathered tile
                        for ki in range(2):
                            nc.tensor.matmul(
                                ar, MT[:, coff + ci, ki, :],
                                WSDQ[:, ki, :], start=(ki == 0), stop=False)
                        nc.tensor.matmul(
                            ar, ST[32 * j:32 * (j + 1), stpos[c0 + ci], :],
                            esd_src[32 * j:32 * (j + 1), sg, 8:16],
                            start=False, stop=True,
                            tile_position=(32 * j, 0))
                    else:
                        nc.tensor.matmul(
                            ar, ST[32 * j:32 * (j + 1), stpos[c0 + ci], :],
                            esd_src[32 * j:32 * (j + 1), sg, 8:16],
                            start=True, stop=True,
                            tile_position=(32 * j, 0))
                # lr = max(alpha, 0.2*alpha); exf = exp(lr)
                lr = sp.tile([128, NCT_MAX, 8], F32, tag="lr")
                apv = zt[:, EO:EO + 8 * nch].rearrange(
                    "p (c e) -> p c e", c=nch)
                if l == 0:
                    alpha = apv
                else:
                    alpha = sp.tile([128, NCT_MAX, 8], F32, tag="alpha")
                    nc.vector.tensor_tensor(
                        alpha[:, 0:nch, :],
                        M[:, coff:coff + nch, 128:144].bitcast(BF16),
                        apv, OP.add)
                    alpha = alpha[:, 0:nch, :]
                nc.scalar.activation(lr[:, 0:nch, :], alpha, AF.Copy,
                                     scale=SLOPE)
                nc.vector.tensor_tensor(lr[:, 0:nch, :], lr[:, 0:nch, :],
                                        alpha, OP.max)
                exf = wp.tile([128, NCT_MAX, 8], F32, tag="exf",
                              name=f"exf{l}_{s}")
                nc.scalar.activation(exf[:, 0:nch, :], lr[:, 0:nch, :],
                                     AF.Exp)
                exq = sp.tile([128, NCT_MAX, 8], FP8, tag="exq")
                nc.scalar.activation(exq[:, 0:nch, :], exf[:, 0:nch, :],
                                     AF.Copy)
                return zt, exf, exq

            def stage_b_den(l, s, zt, exq):
                """den accumulation (col-tiled to partitions 32j..)."""
                nch = int(nct[s])
                c0 = int(slot_c0[s])
                j = s % 4
                for ci in range(nch):
                    nc.tensor.matmul(
                        zt[32 * j:32 * (j + 1), DO:DO + 8],
                        OH[:, c0 + ci, :], exq[:, ci, :],
                        start=(ci == 0), stop=(ci == nch - 1),
                        tile_position=(0, 32 * j))

            def stage_b_rdn(l, s, zt):
                j = s % 4
                rdn = sp.tile([128, 8], BF16, tag="rdn")
                dmx = sp.tile([128, 8], F32, tag="dmx")
                nc.vector.tensor_scalar_max(
                    dmx[32 * j:32 * (j + 1), :],
                    zt[32 * j:32 * (j + 1), DO:DO + 8], 1e-30)
                with nc.allow_low_precision(reason="bf16 1/den is ample"):
                    nc.vector.reciprocal(rdn[32 * j:32 * (j + 1), :],
                                         dmx[32 * j:32 * (j + 1), :])
                return rdn

            def stage_b_rd(l, s, zt, rdn):
                """RD: rden[dst] per edge via 32-row-strip matmuls."""
                nch = int(nct[s])
                c0 = int(slot_c0[s])
                j = s % 4
                for ci in range(nch):
                    nc.tensor.matmul(
                        zt[:, RO + 8 * ci:RO + 8 * ci + 8],
                        ST[32 * j:32 * (j + 1), stpos[c0 + ci], :],
                        rdn[32 * j:32 * (j + 1), :],
                        start=True, stop=True, tile_position=(32 * j, 0))

            def stage_c_s8(l, s, zt, exf):
                nch = int(nct[s])
                c0 = int(slot_c0[s])
                coefq = sp.tile([128, NCT_MAX, 8], FP8, tag="coefq")
                nc.vector.tensor_tensor(
                    coefq[:, 0:nch, :], exf[:, 0:nch, :],
                    zt[:, RO:RO + 8 * nch].rearrange(
                        "p (c e) -> p c e", c=nch),
                    OP.mult)
                # S8 = onehot * coef (broadcast both)
                S8 = wp.tile([128, NCT_MAX, H, TILE], FP8, tag="S8")
                nc.vector.tensor_tensor(
                    S8[:, 0:nch, :, :],
                    OH[:, c0:c0 + nch, None, :].broadcast_to(
                        (128, nch, H, TILE)),
                    coefq[:, 0:nch, :, None].broadcast_to(
                        (128, nch, H, TILE)),
                    OP.mult)
                return S8

            def stage_c_agg(l, s, M, coff, zt, S8, zgs):
                nch = int(nct[s])
                nxh = 2 if l == 0 else 1
                zts = [zt]
                if l == 0:
                    zts.append(pout.tile([128, GSLOT * TILE], F32,
                                         tag="outp", name=f"zb{l}_{s}"))
                for ci in range(nch):
                    st_, sp_ = (ci == 0), (ci == nch - 1)
                    for xh in range(nxh):
                        nc.tensor.matmul(zts[xh][:, 0:ZO],
                                         M[:, coff + ci,
                                           xh * 128:(xh + 1) * 128],
                                         S8[:, ci, :, :],
                                         start=st_, stop=sp_)
                jg = s % GSLOT
                for xh in range(nxh):
                    nc.scalar.activation(
                        zgs[xh][:, :, jg, :],
                        zts[xh][:, 0:ZO].rearrange("p (h d) -> p h d", h=H),
                        AF.Copy)

            def evict_group(l, grp, zgs):
                """W matmul + BN for 256 dst; emit shard rows / a3."""
                outp = pout.tile([128, GSLOT * TILE], F32, tag="outp")
                nxh = 2 if l == 0 else 1
                nmm = H * nxh
                k = 0
                for h in range(H):
                    for xh in range(nxh):
                        wsl = WL[:, (xh if l == 0 else l + 1), h, :]
                        nc.tensor.matmul(outp[:], wsl,
                                         zgs[xh][:, h, :, :],
                                         start=(k == 0), stop=(k == nmm - 1))
                        k += 1
                t1 = wp.tile([128, GSLOT * TILE], F32, tag="t1")
                nc.scalar.activation(t1[:], outp[:], AF.Relu,
                                     bias=BN[:, 3 * l:3 * l + 1])
                act = wp.tile([128, GSLOT * TILE], BF16, tag="act")
                nc.vector.tensor_scalar(
                    out=act[:], in0=t1[:],
                    scalar1=BN[:, 3 * l + 1:3 * l + 2],
                    scalar2=BN[:, 3 * l + 2:3 * l + 3],
                    op0=OP.mult, op1=OP.add)
                if l < 2:
                    esd_dst = ESD[(l + 1) % 2]
                    row = wp.tile([128, 2, ROW], FP8, tag="row")
                    for j in range(2):
                        acol = act[:, j * 128:(j + 1) * 128]
                        ptp = ptr.tile([128, 128], BF16, tag="ptp")
                        nc.tensor.transpose(ptp[:], acol, IDT[:])
                        nc.vector.tensor_copy(row[:, j, 0:128], ptp[:])
                        pe = pscr.tile([128, 16], F32, tag="scr")
                        nc.tensor.matmul(pe[:], acol, WSD[:, l + 2, :],
                                         start=True, stop=True)
                        nc.vector.tensor_copy(esd_dst[:, 2 * grp + j, :],
                                              pe[:])
                        nc.vector.tensor_copy(
                            row[:, j, 128:144].bitcast(BF16),
                            esd_dst[:, 2 * grp + j, 0:8])
                        nc.sync.dma_start(
                            shard[l][grp * 256 + j * 128:
                                     grp * 256 + (j + 1) * 128, :],
                            row[:, j, :])
                else:
                    c0, c1 = grp * 256, min((grp + 1) * 256, PER_CORE)
                    nc.vector.tensor_copy(a3[:, c0:c1], act[:, 0:c1 - c0])
                    for g in range(8):
                        if (g0[g + 1] - 1) // 256 == grp:
                            gm = sp.tile([128, 1], F32, tag="gm")
                            nc.vector.tensor_reduce(
                                gm[:], a3[:, g0[g]:g0[g + 1]],
                                mybir.AxisListType.X, OP.max)
                            nc.vector.tensor_copy(gmpb[:, g:g + 1], gm[:])
                            ga = sp.tile([128, 1], F32, tag="ga")
                            nc.vector.tensor_reduce(
                                ga[:], a3[:, g0[g]:g0[g + 1]],
                                mybir.AxisListType.X, OP.add)
                            nc.vector.tensor_scalar_mul(ga[:], ga[:],
                                                        1.0 / cnt[g])
                            nc.vector.tensor_copy(gapb[:, g:g + 1], ga[:])

            # ---- layers: 3-stage pipeline (A, B lag-1, C lag-2) --------
            # piece starts: layers 1/2 use single-slot tier-split pieces
            # for the first 4 slots (overlap the AllGather tail)
            pstarts = []
            for l in range(3):
                d_ = {}
                if l == 0:
                    for p in range(0, NSLOT, PIECE):
                        d_[p] = min(p + PIECE, NSLOT)
                else:
                    for p in range(4):
                        d_[p] = p + 1
                    for p in range(4, NSLOT, PIECE):
                        d_[p] = min(p + PIECE, NSLOT)
                pstarts.append(d_)
            for l in range(3):
                nxh = 2 if l == 0 else 1
                zgrps = {}
                st = {}
                Mp = MTp = None
                pcur = 0
                for s in range(NSLOT + 2):
                    if 1 <= s <= NSLOT:
                        ps = s - 1
                        stage_b_den(l, ps, st[ps][2], st[ps][4])
                        st[ps] = st[ps][:5] + (stage_b_rdn(l, ps, st[ps][2]),)
                    if s >= 2:
                        ps = s - 2
                        S8 = stage_c_s8(l, ps, st[ps][2], st[ps][3])
                        if ps % GSLOT == 0:
                            zgrps[ps // GSLOT] = [
                                zgp.tile([128, H, GSLOT, TILE], BF16,
                                         tag=f"zg{xh}",
                                         name=f"zg{l}_{ps}_{xh}")
                                for xh in range(nxh)]
                    if s < NSLOT:
                        if s in pstarts[l]:
                            Mp, MTp = gather_piece(l, s, pstarts[l][s])
                            pcur = s
                        coff = int(slot_c0[s] - slot_c0[pcur])
                        zt, exf, exq = stage_a(l, s, Mp, MTp, coff)
                        st[s] = (Mp, coff, zt, exf, exq)
                    if 1 <= s <= NSLOT:
                        ps = s - 1
                        stage_b_rd(l, ps, st[ps][2], st[ps][5])
                    if s >= 2:
                        ps = s - 2
                        stage_c_agg(l, ps, st[ps][0], st[ps][1], st[ps][2],
                                    S8, zgrps[ps // GSLOT])
                        del st[ps]
                        if ps % GSLOT == GSLOT - 1:
                            grp = ps // GSLOT
                            evict_group(l, grp, zgrps.pop(grp))
                            if l < 2:
                                if grp == 1:
                                    emit_ag(l + 1, 0)
                                elif grp == 3:
                                    emit_ag(l + 1, 1)
                                elif grp == 4:
                                    emit_ag(l + 1, 2)
            # ---- readout ------------------------------------------------
            phg = pscr.tile([128, 8], F32, tag="scr")
            nc.tensor.matmul(phg[:], L0W[:, 0, :], gmpb[:], start=True,
                             stop=False)
            nc.tensor.matmul(phg[:], L0W[:, 1, :], gapb[:], start=False,
                             stop=True)
            hg = pp.tile([128, 8], BF16)
            nc.scalar.activation(hg[:], phg[:], AF.Relu, bias=L0B[:])
            pnw = pscr.tile([128, 8], F32, tag="scr")
            nc.tensor.matmul(pnw[:], LNW[:, 0, :], XR[:, 0, :], start=True,
                             stop=False)
            nc.tensor.matmul(pnw[:], LNW[:, 1, :], XR[:, 1, :], start=False,
                             stop=True)
            nw = pp.tile([128, 8], BF16)
            nc.scalar.activation(nw[:], pnw[:], AF.Relu, bias=LNB[:])
            pfin = pscr.tile([8, 1], F32, tag="scr")
            nc.tensor.matmul(pfin[:], hg[:], L1W[:, 0, :], start=True,
                             stop=False)
            nc.tensor.matmul(pfin[:], nw[:], L1W[:, 1, :], start=False,
                             stop=True)
            fin = pp.tile([8, 1], F32)
            nc.scalar.activation(fin[:], pfin[:], AF.Sigmoid, bias=L1B[:])
            nc.sync.dma_start(out_t[:], fin[:])
    nc.compile()
    return nc


def kernel(x, edge_index, batch,
           W1, as1, ad1, b1, g1, bb1, m1, v1,
           W2, as2, ad2, b2, g2, bb2, m2, v2,
           W3, as3, ad3, b3, g3, bb3, m3, v3,
           lnW, lnb, l0W, l0b, l1W, l1b):
    x = np.asarray(x, np.float32)
    edge_index = np.asarray(edge_index)
    batch = np.asarray(batch)
    Ws = [np.asarray(w, np.float64) for w in (W1, W2, W3)]
    ass = [np.asarray(a, np.float64) for a in (as1, as2, as3)]
    ads = [np.asarray(a, np.float64) for a in (ad1, ad2, ad3)]
    bs = [np.asarray(a, np.float32) for a in (b1, b2, b3)]
    gs = [np.asarray(a, np.float32) for a in (g1, g2, g3)]
    bbs = [np.asarray(a, np.float32) for a in (bb1, bb2, bb3)]
    ms = [np.asarray(a, np.float32) for a in (m1, m2, m3)]
    vs = [np.asarray(a, np.float32) for a in (v1, v2, v3)]

    src = np.concatenate([edge_index[0], np.arange(N)]).astype(np.int64)
    dst = np.concatenate([edge_index[1], np.arange(N)]).astype(np.int64)

    # ---- degree-balanced within-graph node->column permutation --------
    # Slot edge-loads should sit just under multiples of 128 to minimize
    # chunk padding (padded idxs cost real gather time).
    import bisect
    deg = np.bincount(dst, minlength=N).astype(np.int64)
    bnds0 = np.searchsorted(batch, np.arange(G + 1))
    Tk = np.array([deg[k * PER_CORE:(k + 1) * PER_CORE].sum()
                   for k in range(NCORES)])
    n4 = max(0, int(np.ceil((Tk.max() + 200 - NSLOT * 376) / 128.0)))
    n4 = min(n4, NSLOT)
    cap = np.full(NSLOT, 376, np.int64)
    if n4 > 0:
        for s in np.linspace(0, NSLOT - 1, n4).astype(int):
            cap[s] = 504
    col_of = np.zeros(N, np.int64)
    for k in range(NCORES):
        gidx = 8 * k
        pool_deg, pool_node = [], []
        rem = float(cap[0])
        for col in range(PER_CORE):
            gcol = k * PER_CORE + col
            s = col // TILE
            if col % TILE == 0 and col > 0:
                rem = float(cap[s])
            while gidx < 8 * (k + 1) and int(bnds0[gidx]) == gcol:
                lo, hi = int(bnds0[gidx]), int(bnds0[gidx + 1])
                order_g = np.argsort(deg[lo:hi], kind="stable")
                pool_node = list(lo + order_g)
                pool_deg = list(deg[lo:hi][order_g])
                gidx += 1
            m = min(TILE * (s + 1), PER_CORE) - col
            tgt = rem / max(m, 1)
            i = bisect.bisect_left(pool_deg, tgt)
            if i >= len(pool_deg):
                i = len(pool_deg) - 1
            elif i > 0 and (pool_deg[i] - tgt) > (tgt - pool_deg[i - 1]):
                i -= 1
            nsel = pool_node.pop(i)
            rem -= pool_deg.pop(i)
            col_of[nsel] = gcol
    node_at = np.zeros(N, np.int64)
    node_at[col_of] = np.arange(N)

    core = dst // PER_CORE
    dloc = col_of[dst] - core * PER_CORE
    slot = dloc // TILE
    din = dloc - slot * TILE

    # full-table row layout: AG-group-major, then core, then slot, then din
    grp_of_slot = np.zeros(NSLOT, np.int64)
    gb = np.zeros(3, np.int64)
    gs0 = np.zeros(3, np.int64)
    gn = np.zeros(3, np.int64)
    base = 0
    for gi_, (s0, s1) in enumerate(AG_GROUPS):
        grp_of_slot[s0:s1] = gi_
        gb[gi_], gs0[gi_], gn[gi_] = base, s0, s1 - s0
        base += (s1 - s0) * TILE * NCORES
    assert base == NFULL

    nodes_c = col_of                       # node -> global column
    k_n = nodes_c // PER_CORE
    dl_n = nodes_c - k_n * PER_CORE
    s_n = dl_n // TILE
    d_n = dl_n - s_n * TILE
    gi_n = grp_of_slot[s_n]
    frow = gb[gi_n] + k_n * gn[gi_n] * TILE + (s_n - gs0[gi_n]) * TILE + d_n
    assert frow.max() < NFULL and len(np.unique(frow)) == N

    # order edges by (core, slot, tier of src row)
    tier = (frow[src] >= 4096).astype(np.int64) + (frow[src] >= 8192)
    order = np.lexsort((tier, slot + 100 * core))
    srco = src[order]
    coreo = core[order]
    sloto = slot[order]
    dino = din[order]

    cnts = np.zeros((NCORES, NSLOT), np.int64)
    np.add.at(cnts, (coreo, sloto), 1)
    nct = np.maximum(
        np.ceil(cnts.max(axis=0) / 128).astype(np.int64), 1)
    nchunk = int(nct.sum())
    slot_c0 = np.concatenate([[0], np.cumsum(nct)]).astype(int)

    # st table position per chunk: phase j = s%4 rows; sequential per phase
    stpos = np.zeros(nchunk, np.int64)
    phase_pos = [0, 0, 0, 0]
    for s in range(NSLOT):
        j = s % 4
        for ci in range(nct[s]):
            stpos[slot_c0[s] + ci] = phase_pos[j]
            phase_pos[j] += 1
    CH4 = max(phase_pos)

    # per-edge chunk-local position (edges already (core,slot)-grouped)
    offs = np.zeros((NCORES, NSLOT), np.int64)
    flat = cnts.flatten()
    offs_flat = np.concatenate([[0], np.cumsum(flat)[:-1]])
    offs = offs_flat.reshape(NCORES, NSLOT)
    pos = np.arange(len(srco)) - offs[coreo, sloto]
    ch_of = slot_c0[sloto] + pos // 128
    e_of = pos % 128

    gsrc = np.zeros((NCORES, nchunk * 128), np.int16)
    gsrc[coreo, ch_of * 128 + e_of] = frow[srco]
    oh_np = np.zeros((NCORES, 128, nchunk, TILE), np.float32)
    oh_np[coreo, e_of, ch_of, dino] = 1.0
    st_np = np.zeros((NCORES, 128, CH4, 128), np.float32)
    st_np[coreo, 32 * (sloto % 4) + dino, stpos[ch_of], e_of] = 1.0

    gi = gsrc.reshape(NCORES, nchunk * 8, 16).transpose(0, 2, 1)
    gi = np.ascontiguousarray(np.tile(gi, (1, 8, 1)))

    # per-piece gather trim: skip trailing pad idxs of the piece's last slot
    PIECE = 2
    trim = {}
    for p in range(NSLOT):
        for last in (p, min(p + PIECE, NSLOT) - 1):
            nch_piece = int(slot_c0[last + 1] - slot_c0[p])
            lastfill = int(cnts[:, last].max() - (nct[last] - 1) * 128)
            lastfill = max(lastfill, 1)
            trim[(p, last + 1)] = (nch_piece - 1) * 128 + lastfill

    # chunk_lim: max row index any core's edges in this chunk may touch
    t0c = np.zeros((NCORES, NSLOT), np.int64)
    t1c = np.zeros((NCORES, NSLOT), np.int64)
    tiero = tier[order]
    np.add.at(t0c, (coreo, sloto), (tiero == 0).astype(np.int64))
    np.add.at(t1c, (coreo, sloto), (tiero <= 1).astype(np.int64))
    chunk_lim = np.full(nchunk, NFULL, np.int64)
    for s in range(NSLOT):
        for ci in range(int(nct[s])):
            e_end = (ci + 1) * 128
            if all(e_end <= t0c[k, s] for k in range(NCORES)):
                chunk_lim[slot_c0[s] + ci] = 4096
            elif all(e_end <= t1c[k, s] for k in range(NCORES)):
                chunk_lim[slot_c0[s] + ci] = 8192

    # layer-0 pregathered tiles (host-side static gather of input x)
    x8 = _f8(np.asarray(x, np.float32))                 # [N, 256]
    x8z = np.zeros((N + 1, 256), ml_dtypes.float8_e4m3)
    x8z[:N] = x8
    mx0_np = np.zeros((NCORES, 128, nchunk, 256), ml_dtypes.float8_e4m3)
    mx0_np[coreo, e_of, ch_of, :] = x8z[srco]
    mxt0_np = np.zeros((NCORES, 128, nchunk, 2, 128),
                       ml_dtypes.float8_e4m3)
    tmp = np.ascontiguousarray(x8z[srco].reshape(-1, 2, 128))
    mxt0_np[coreo, :, ch_of, :, e_of] = tmp.transpose(0, 2, 1)

    # weights
    wl = np.zeros((128, 4, H, 128), np.float64)
    wsd = np.zeros((128, 4, 16), np.float64)
    for li in range(3):
        W = Ws[li]
        ws = np.stack([W[:, h * C:(h + 1) * C] @ ass[li][h]
                       for h in range(H)], 1)
        wd = np.stack([W[:, h * C:(h + 1) * C] @ ads[li][h]
                       for h in range(H)], 1)
        if li == 0:
            for ki in range(2):
                for h in range(H):
                    wl[:, ki, h, :] = W[ki * 128:(ki + 1) * 128,
                                        h * C:(h + 1) * C]
                wsd[:, ki, 0:8] = ws[ki * 128:(ki + 1) * 128]
                wsd[:, ki, 8:16] = wd[ki * 128:(ki + 1) * 128]
        else:
            for h in range(H):
                wl[:, li + 1, h, :] = W[:, h * C:(h + 1) * C]
            wsd[:, li + 1, 0:8] = ws
            wsd[:, li + 1, 8:16] = wd

    wsdq = _f8(wsd[:, 0:2, 0:8])

    bn = np.zeros((128, 9), np.float32)
    for li in range(3):
        r = 1.0 / np.sqrt(vs[li] + EPS)
        bn[:, 3 * li + 0] = 8.0 * bs[li]
        bn[:, 3 * li + 1] = gs[li] * r / 8.0
        bn[:, 3 * li + 2] = bbs[li] - ms[li] * gs[li] * r

    bnds = np.searchsorted(batch, np.arange(G + 1))
    assert bnds[G] == N
    for k in range(1, NCORES):
        assert bnds[8 * k] == PER_CORE * k, "graphs must align to cores"
    g0 = [int(bnds[g]) for g in range(9)]
    cnt = [float(bnds[g + 1] - bnds[g]) for g in range(8)]
    for k in range(1, NCORES):
        for g in range(9):
            assert int(bnds[8 * k + g]) - PER_CORE * k == g0[g]
    root = bnds[:G]
    x64 = np.asarray(x, np.float64)
    xrT = x64[root].T.reshape(2, 128, G)

    l0w = _bf(np.asarray(l0W).reshape(2, 128, 128).transpose(1, 0, 2))
    lnw = _bf(np.asarray(lnW).reshape(2, 128, 128).transpose(1, 0, 2))
    l1w = _bf(np.asarray(l1W).reshape(2, 128, 1).transpose(1, 0, 2))

    nc = build_nc([int(v) for v in nct], [int(v) for v in stpos], g0, cnt,
                  trim, chunk_lim)

    in_maps = []
    for k in range(NCORES):
        xk = x64[node_at[k * PER_CORE:(k + 1) * PER_CORE]]   # [1250, 256]
        xTk = np.zeros((128, 2, RPAD), ml_dtypes.bfloat16)
        xTk[:, :, :PER_CORE] = _bf(np.ascontiguousarray(
            xk.T.reshape(2, 128, PER_CORE).transpose(1, 0, 2)))
        in_maps.append(dict(
            mx0=np.ascontiguousarray(mx0_np[k]),
            mxt0=np.ascontiguousarray(mxt0_np[k]),
            wsdq=wsdq, xT=xTk,
            oh=_f8(oh_np[k]), st=_bf(st_np[k]), gi=gi[k],
            wl=_bf(wl), wsd=_bf(wsd), bn=bn,
            ident=_bf(np.eye(128)),
            xrootT=_bf(np.ascontiguousarray(
                xrT[:, :, 8 * k:8 * k + 8].transpose(1, 0, 2))),
            l0w=l0w, lnw=lnw, l1w=l1w,
            l0b=np.asarray(l0b, np.float32).reshape(128, 1),
            lnb=np.asarray(lnb, np.float32).reshape(128, 1),
            l1b=np.broadcast_to(np.asarray(l1b, np.float32), (8, 1)).copy(),
        ))
    global LAST_RESULT, LAST_NC, LAST_INMAPS
    LAST_NC, LAST_INMAPS = nc, in_maps
    res = run_bass_kernel_spmd(nc, in_maps, core_ids=list(range(NCORES)))
    LAST_RESULT = res
    out = np.concatenate([res.results[k]["out"] for k in range(NCORES)], 0)
    return out.astype(np.float32)


# revision 45
# speedup vs baseline: 1.0151x; 1.0151x over previous
"""Trainium2 Bass kernel for 3-layer GAT + pooling readout (nn_GNN_7653631722064).

v4.5 (~450us, vs 900us v3 baseline). Key ideas over v3:
- Aggregate PRE-transform features: sum_e coef_e*(x_e @ W) =
  (sum_e coef_e*x_e) @ W, so gather x[src] rows (128B fp8 x + 16B bf16
  es, 256B elems) instead of h[src] (1280B): 5x less gather traffic,
  5x smaller AllGather tables; W applied per head at evict over
  256-dst groups, head-sum + BN fused into one psum accumulation.
- Layer 0 needs NO gather at all: host pre-gathers per-edge x tiles
  (plain input layout) in both [e,x] and [x,e] orientations; es0 per
  edge comes from a matmul on the transposed tile that accumulates
  es0[src]+ed0[dst] straight into the alpha psum region.
- dma_gather costs ~6-8ns/idx of serial gpsimd time -> idx count is
  the wall: degree-balanced within-graph node placement packs slot
  edge counts just under multiples of 128 (26% -> ~5% chunk padding),
  trailing pad idxs trimmed via num_idxs, gathers striped across 4
  SWDGE queues (num_swdge_queues=4 gives ~2.3x gather throughput).
- 32-dst slots; per 128-edge chunk ONE wide aggregation matmul
  (lhsT = gathered Mx stationary, rhs = coef-scaled one-hot S8
  [e, 8h*32d]); ED/RD (ed[dst], rden[dst] per edge) via 32-row-strip
  tile_position matmuls; den via col-tiled matmul; coefficients are
  normalized BEFORE aggregation (coef = exp(lrelu)*rden[dst]) so no
  partition-broadcast is ever needed.
- 3-stage software pipeline per slot (A: gather+alpha/exp; B lag-1:
  den+recip+RD; C lag-2: coef+S8+agg) keeps the PE from stalling on
  vector/scalar round-trips; all per-slot psum state (zT, alpha, RDp,
  den) lives in one 2KB psum bank; first 4 slots of layers 1/2 use
  tier-split gathers bounded to already-landed AllGather groups.
"""
import sys

sys.path.insert(0, "/opt/trn_rl_repo")

import numpy as np
import ml_dtypes

import concourse.bass as bass
import concourse.tile as tile
from concourse import bacc, mybir
from concourse.bass_utils import run_bass_kernel_spmd

BF16 = mybir.dt.bfloat16
FP8 = mybir.dt.float8e4
F32 = mybir.dt.float32
I16 = mybir.dt.int16
AF = mybir.ActivationFunctionType
OP = mybir.AluOpType

N, E, IN, H, C, G = 10000, 120000, 256, 8, 128, 64
NCORES = 8
TILE = 32                      # dst nodes per slot
NSLOT = 40                     # slots per core (40*32=1280 >= 1250)
PER_CORE = 1250
RPAD = NSLOT * TILE            # padded rows per core (1280)
NFULL = RPAD * NCORES          # 10240 rows in full tables
ROW = 256                      # L1/2 row bytes: 128 fp8 x + 16B es + pad
GSLOT = 8                      # slots per evict group (256 dst)
NGRP = NSLOT // GSLOT          # 5 groups
AG_GROUPS = [(0, 16), (16, 32), (32, 40)]   # slot ranges per AllGather
EPS = 1e-5
SLOPE = 0.2


def _bf(a):
    return np.asarray(a, dtype=ml_dtypes.bfloat16)


def _f8(a):
    return np.asarray(a, dtype=ml_dtypes.float8_e4m3)


def build_nc(nct, stpos, g0, cnt, trim, chunk_lim):
    nchunk = int(sum(nct))
    NCT_MAX = int(max(nct))
    slot_c0 = np.concatenate([[0], np.cumsum(nct)]).astype(int)
    CH4 = int(max(stpos)) + 1
    nc = bacc.Bacc(None, target_bir_lowering=False, debug=False,
                   num_devices=NCORES, name="gatx", num_swdge_queues=4)

    mx0_in = nc.dram_tensor("mx0", [128, nchunk, 256], FP8,
                            kind="ExternalInput")
    mxt0_in = nc.dram_tensor("mxt0", [128, nchunk, 2, 128], FP8,
                             kind="ExternalInput")
    wsdq_in = nc.dram_tensor("wsdq", [128, 2, 8], FP8, kind="ExternalInput")
    xT_in = nc.dram_tensor("xT", [128, 2, RPAD], BF16, kind="ExternalInput")
    oh_in = nc.dram_tensor("oh", [128, nchunk, TILE], FP8, kind="ExternalInput")
    st_in = nc.dram_tensor("st", [128, CH4, 128], BF16, kind="ExternalInput")
    gi_in = nc.dram_tensor("gi", [128, nchunk * 8], I16, kind="ExternalInput")
    wl_in = nc.dram_tensor("wl", [128, 4, H, 128], BF16, kind="ExternalInput")
    wsd_in = nc.dram_tensor("wsd", [128, 4, 16], BF16, kind="ExternalInput")
    bn_in = nc.dram_tensor("bn", [128, 9], F32, kind="ExternalInput")
    id_in = nc.dram_tensor("ident", [128, 128], BF16, kind="ExternalInput")
    xr_in = nc.dram_tensor("xrootT", [128, 2, 8], BF16, kind="ExternalInput")
    l0w_in = nc.dram_tensor("l0w", [128, 2, 128], BF16, kind="ExternalInput")
    lnw_in = nc.dram_tensor("lnw", [128, 2, 128], BF16, kind="ExternalInput")
    l1w_in = nc.dram_tensor("l1w", [128, 2, 1], BF16, kind="ExternalInput")
    l0b_in = nc.dram_tensor("l0b", [128, 1], F32, kind="ExternalInput")
    lnb_in = nc.dram_tensor("lnb", [128, 1], F32, kind="ExternalInput")
    l1b_in = nc.dram_tensor("l1b", [8, 1], F32, kind="ExternalInput")
    out_t = nc.dram_tensor("out", [8, 1], F32, kind="ExternalOutput")

    warm_in = nc.dram_tensor("warm_in", [8, 128], FP8, kind="Internal")
    warm_out = nc.dram_tensor("warm_out", [64, 128], FP8, kind="Internal",
                              addr_space="Shared")
    shard = [nc.dram_tensor(f"shard{l}", [RPAD, ROW], FP8, kind="Internal")
             for l in (1, 2)]
    fullx = [nc.dram_tensor(f"full{l}", [NFULL, ROW], FP8, kind="Internal",
                            addr_space="Shared")
             for l in (1, 2)]

    def full_t(l):
        return fullx[l - 1]

    with tile.TileContext(nc) as tc:
        with (
            tc.tile_pool(name="persist", bufs=1) as pp,
            tc.tile_pool(name="work", bufs=4) as wp,
            tc.tile_pool(name="mbuf", bufs=7) as mp,
            tc.tile_pool(name="small", bufs=4) as sp,
            tc.tile_pool(name="zg", bufs=3) as zgp,
            tc.tile_pool(name="pz", bufs=4, space="PSUM") as pz,
            tc.tile_pool(name="pout", bufs=2, space="PSUM") as pout,
            tc.tile_pool(name="ptr", bufs=1, space="PSUM") as ptr,
            tc.tile_pool(name="pscr", bufs=1, space="PSUM") as pscr,
        ):
            XT = pp.tile([128, 2, RPAD], BF16)
            OH = pp.tile([128, nchunk, TILE], FP8)
            ST = pp.tile([128, CH4, 128], BF16)
            GI = pp.tile([128, nchunk * 8], I16)
            WL = pp.tile([128, 4, H, 128], BF16)
            WSD = pp.tile([128, 4, 16], BF16)
            WSDQ = pp.tile([128, 2, 8], FP8)
            BN = pp.tile([128, 9], F32)
            IDT = pp.tile([128, 128], BF16)
            XR = pp.tile([128, 2, 8], BF16)
            L0W = pp.tile([128, 2, 128], BF16)
            LNW = pp.tile([128, 2, 128], BF16)
            L1W = pp.tile([128, 2, 1], BF16)
            L0B = pp.tile([128, 1], F32)
            LNB = pp.tile([128, 1], F32)
            L1B = pp.tile([8, 1], F32)
            ESD = [pp.tile([128, 10, 16], BF16, name=f"esd{i}")
                   for i in range(2)]
            a3 = pp.tile([128, PER_CORE], BF16)
            gmpb = pp.tile([128, 8], BF16)
            gapb = pp.tile([128, 8], BF16)
            for t, src_ in [(XT, xT_in), (OH, oh_in), (ST, st_in),
                            (GI, gi_in), (WL, wl_in), (WSD, wsd_in),
                            (WSDQ, wsdq_in),
                            (BN, bn_in), (IDT, id_in), (XR, xr_in),
                            (L0W, l0w_in), (LNW, lnw_in), (L1W, l1w_in),
                            (L0B, l0b_in), (LNB, lnb_in), (L1B, l1b_in)]:
                nc.sync.dma_start(t[:], src_[:])

            wtile = sp.tile([8, 128], FP8, tag="warm")
            nc.vector.memset(wtile[:], 0.0)
            nc.sync.dma_start(warm_in[:], wtile[:])
            nc.gpsimd.collective_compute(
                "AllGather", OP.bypass,
                replica_groups=[list(range(NCORES))],
                ins=[warm_in[:].opt()], outs=[warm_out[:].opt()])

            # ---- es0/ed0 for own nodes (feeds ED strips of layer 0) ----
            for g in range(10):
                pe = pscr.tile([128, 16], F32, tag="scr")
                nc.tensor.matmul(pe[:], XT[:, 0, g * 128:(g + 1) * 128],
                                 WSD[:, 0, :], start=True, stop=False)
                nc.tensor.matmul(pe[:], XT[:, 1, g * 128:(g + 1) * 128],
                                 WSD[:, 1, :], start=False, stop=True)
                nc.vector.tensor_copy(ESD[0][:, g, :], pe[:])

            def emit_ag(l, gi_):
                s0, s1 = AG_GROUPS[gi_]
                r0, r1 = s0 * TILE, s1 * TILE
                shd = shard[l - 1]
                f0 = r0 * NCORES
                nc.gpsimd.collective_compute(
                    "AllGather", OP.bypass,
                    replica_groups=[list(range(NCORES))],
                    ins=[shd[r0:r1, :].opt()],
                    outs=[full_t(l)[f0:f0 + (r1 - r0) * NCORES, :].opt()])

            # ------------------------------------------------------------
            # per-slot psum bank layout (f32 elems):
            #   [0:256]   zT accumulation [x, h*d]
            #   [256:288] alpha (L0: es0+ed accumulated; L1/2: EDp)
            #   [288:320] RDp  (rden[dst] per edge, [nch, 8])
            #   [320:328] den  (on partitions 32j..32j+32)
            ZO = H * TILE
            EO = ZO
            RO = EO + 8 * NCT_MAX
            DO = RO + 8 * NCT_MAX
            assert DO + 8 <= 512
            PIECE = 2
            CMAX = int(max(slot_c0[p + PIECE] - slot_c0[p]
                           for p in range(0, NSLOT, PIECE)))

            # scrub gather buffers once so pad positions never hold
            # uninitialized SBUF bytes (exp(garbage) -> inf -> NaN)
            for r in range(7):
                Mw = mp.tile([128, CMAX, ROW], FP8, tag="M1",
                             name=f"Mws{r}")
                nc.vector.memset(Mw[:], 0.0)

            def gather_piece(l, p, s_end):
                c0 = int(slot_c0[p])
                c1 = int(slot_c0[s_end])
                nch = c1 - c0
                if l == 0:
                    M = mp.tile([128, CMAX, 256], FP8, tag="M0",
                                name=f"M0_{p}")
                    nc.sync.dma_start(M[:, 0:nch, :], mx0_in[:, c0:c1, :])
                    MT = mp.tile([128, CMAX, 2, 128], FP8, tag="MT0",
                                 name=f"MT0_{p}")
                    nc.sync.dma_start(MT[:, 0:nch, :, :],
                                      mxt0_in[:, c0:c1, :, :])
                    return M, MT
                M = mp.tile([128, CMAX, ROW], FP8, tag="M1",
                            name=f"M1_{l}_{p}")
                if s_end - p == 1:
                    # tier-split: sub-gathers limited to landed AG groups
                    subs = []
                    cl = [int(chunk_lim[c]) for c in range(c0, c1)]
                    i = 0
                    while i < nch:
                        k2 = i
                        while k2 < nch and cl[k2] == cl[i]:
                            k2 += 1
                        subs.append((i, k2, cl[i]))
                        i = k2
                    for (i, k2, lim) in subs:
                        ni = (k2 - i) * 128
                        if k2 == nch:
                            ni = int(trim[(p, s_end)]) - i * 128
                        nc.gpsimd.dma_gather(
                            out_ap=M[:, i:k2, :], in_ap=full_t(l)[0:lim],
                            idxs_ap=GI[:, (c0 + i) * 8:(c0 + k2) * 8],
                            num_idxs=ni, num_idxs_reg=ni,
                            elem_size=ROW, elem_step=ROW,
                            queue_num=p % 4)
                    return M, None
                nc.gpsimd.dma_gather(
                    out_ap=M[:, 0:nch, :], in_ap=full_t(l)[0:NFULL],
                    idxs_ap=GI[:, c0 * 8:c1 * 8],
                    num_idxs=int(trim[(p, s_end)]),
                    num_idxs_reg=int(trim[(p, s_end)]),
                    elem_size=ROW, elem_step=ROW,
                    queue_num=(p // PIECE) % 4)
                return M, None

            def stage_a(l, s, M, MT, coff):
                """alpha (es[src]+ed[dst]) + exp chain."""
                esd_src = ESD[l % 2]
                nch = int(nct[s])
                c0 = int(slot_c0[s])
                j = s % 4
                sg = s // 4
                zt = pz.tile([128, 512], F32, tag="zt", name=f"z{l}_{s}")
                for ci in range(nch):
                    ar = zt[:, EO + 8 * ci:EO + 8 * ci + 8]
                    if l == 0:
                        # es0 = x@ws via transposed pregathered tile
                        for ki in range(2):
                            nc.tensor.matmul(
                                ar, MT[:, coff + ci, ki, :],
                                WSDQ[:, ki, :], start=(ki == 0), stop=False)
                        nc.tensor.matmul(
                            ar, ST[32 * j:32 * (j + 1), stpos[c0 + ci], :],
                            esd_src[32 * j:32 * (j + 1), sg, 8:16],
                            start=False, stop=True,
                            tile_position=(32 * j, 0))
                    else:
                        nc.tensor.matmul(
                            ar, ST[32 * j:32 * (j + 1), stpos[c0 + ci], :],
                            esd_src[32 * j:32 * (j + 1), sg, 8:16],
                            start=True, stop=True,
                            tile_position=(32 * j, 0))
                # lr = max(alpha, 0.2*alpha); exf = exp(lr)
                lr = sp.tile([128, NCT_MAX, 8], F32, tag="lr")
                apv = zt[:, EO:EO + 8 * nch].rearrange(
                    "p (c e) -> p c e", c=nch)
                if l == 0:
                    alpha = apv
                else:
                    alpha = sp.tile([128, NCT_MAX, 8], F32, tag="alpha")
                    nc.vector.tensor_tensor(
                        alpha[:, 0:nch, :],
                        M[:, coff:coff + nch, 128:144].bitcast(BF16),
                        apv, OP.add)
                    alpha = alpha[:, 0:nch, :]
                nc.scalar.activation(lr[:, 0:nch, :], alpha, AF.Copy,
                                     scale=SLOPE)
                nc.vector.tensor_tensor(lr[:, 0:nch, :], lr[:, 0:nch, :],
                                        alpha, OP.max)
                exf = wp.tile([128, NCT_MAX, 8], F32, tag="exf",
                              name=f"exf{l}_{s}")
                nc.scalar.activation(exf[:, 0:nch, :], lr[:, 0:nch, :],
                                     AF.Exp)
                exq = sp.tile([128, NCT_MAX, 8], FP8, tag="exq")
                nc.scalar.activation(exq[:, 0:nch, :], exf[:, 0:nch, :],
                                     AF.Copy)
                return zt, exf, exq

            def stage_b_den(l, s, zt, exq):
                """den accumulation (col-tiled to partitions 32j..)."""
                nch = int(nct[s])
                c0 = int(slot_c0[s])
                j = s % 4
                for ci in range(nch):
                    nc.tensor.matmul(
                        zt[32 * j:32 * (j + 1), DO:DO + 8],
                        OH[:, c0 + ci, :], exq[:, ci, :],
                        start=(ci == 0), stop=(ci == nch - 1),
                        tile_position=(0, 32 * j))

            def stage_b_rdn(l, s, zt):
                j = s % 4
                rdn = sp.tile([128, 8], BF16, tag="rdn")
                dmx = sp.tile([128, 8], F32, tag="dmx")
                nc.vector.tensor_scalar_max(
                    dmx[32 * j:32 * (j + 1), :],
                    zt[32 * j:32 * (j + 1), DO:DO + 8], 1e-30)
                with nc.allow_low_precision(reason="bf16 1/den is ample"):
                    nc.vector.reciprocal(rdn[32 * j:32 * (j + 1), :],
                                         dmx[32 * j:32 * (j + 1), :])
                return rdn

            def stage_b_rd(l, s, zt, rdn):
                """RD: rden[dst] per edge via 32-row-strip matmuls."""
                nch = int(nct[s])
                c0 = int(slot_c0[s])
                j = s % 4
                for ci in range(nch):
                    nc.tensor.matmul(
                        zt[:, RO + 8 * ci:RO + 8 * ci + 8],
                        ST[32 * j:32 * (j + 1), stpos[c0 + ci], :],
                        rdn[32 * j:32 * (j + 1), :],
                        start=True, stop=True, tile_position=(32 * j, 0))

            def stage_c_s8(l, s, zt, exf):
                nch = int(nct[s])
                c0 = int(slot_c0[s])
                coefq = sp.tile([128, NCT_MAX, 8], FP8, tag="coefq")
                nc.vector.tensor_tensor(
                    coefq[:, 0:nch, :], exf[:, 0:nch, :],
                    zt[:, RO:RO + 8 * nch].rearrange(
                        "p (c e) -> p c e", c=nch),
                    OP.mult)
                # S8 = onehot * coef (broadcast both)
                S8 = wp.tile([128, NCT_MAX, H, TILE], FP8, tag="S8")
                nc.vector.tensor_tensor(
                    S8[:, 0:nch, :, :],
                    OH[:, c0:c0 + nch, None, :].broadcast_to(
                        (128, nch, H, TILE)),
                    coefq[:, 0:nch, :, None].broadcast_to(
                        (128, nch, H, TILE)),
                    OP.mult)
                return S8

            def stage_c_agg(l, s, M, coff, zt, S8, zgs):
                nch = int(nct[s])
                nxh = 2 if l == 0 else 1
                zts = [zt]
                if l == 0:
                    zts.append(pout.tile([128, GSLOT * TILE], F32,
                                         tag="outp", name=f"zb{l}_{s}"))
                for ci in range(nch):
                    st_, sp_ = (ci == 0), (ci == nch - 1)
                    for xh in range(nxh):
                        nc.tensor.matmul(zts[xh][:, 0:ZO],
                                         M[:, coff + ci,
                                           xh * 128:(xh + 1) * 128],
                                         S8[:, ci, :, :],
                                         start=st_, stop=sp_)
                jg = s % GSLOT
                for xh in range(nxh):
                    nc.scalar.activation(
                        zgs[xh][:, :, jg, :],
                        zts[xh][:, 0:ZO].rearrange("p (h d) -> p h d", h=H),
                        AF.Copy)

            def evict_group(l, grp, zgs):
                """W matmul + BN for 256 dst; emit shard rows / a3."""
                outp = pout.tile([128, GSLOT * TILE], F32, tag="outp")
                nxh = 2 if l == 0 else 1
                nmm = H * nxh
                k = 0
                for h in range(H):
                    for xh in range(nxh):
                        wsl = WL[:, (xh if l == 0 else l + 1), h, :]
                        nc.tensor.matmul(outp[:], wsl,
                                         zgs[xh][:, h, :, :],
                                         start=(k == 0), stop=(k == nmm - 1))
                        k += 1
                t1 = wp.tile([128, GSLOT * TILE], F32, tag="t1")
                nc.scalar.activation(t1[:], outp[:], AF.Relu,
                                     bias=BN[:, 3 * l:3 * l + 1])
                act = wp.tile([128, GSLOT * TILE], BF16, tag="act")
                nc.vector.tensor_scalar(
                    out=act[:], in0=t1[:],
                    scalar1=BN[:, 3 * l + 1:3 * l + 2],
                    scalar2=BN[:, 3 * l + 2:3 * l + 3],
                    op0=OP.mult, op1=OP.add)
                if l < 2:
                    esd_dst = ESD[(l + 1) % 2]
                    row = wp.tile([128, 2, ROW], FP8, tag="row")
                    for j in range(2):
                        acol = act[:, j * 128:(j + 1) * 128]
                        ptp = ptr.tile([128, 128], BF16, tag="ptp")
                        nc.tensor.transpose(ptp[:], acol, IDT[:])
                        nc.vector.tensor_copy(row[:, j, 0:128], ptp[:])
                        pe = pscr.tile([128, 16], F32, tag="scr")
                        nc.tensor.matmul(pe[:], acol, WSD[:, l + 2, :],
                                         start=True, stop=True)
                        nc.vector.tensor_copy(esd_dst[:, 2 * grp + j, :],
                                              pe[:])
                        nc.vector.tensor_copy(
                            row[:, j, 128:144].bitcast(BF16),
                            esd_dst[:, 2 * grp + j, 0:8])
                        nc.sync.dma_start(
                            shard[l][grp * 256 + j * 128:
                                     grp * 256 + (j + 1) * 128, :],
                            row[:, j, :])
                else:
                    c0, c1 = grp * 256, min((grp + 1) * 256, PER_CORE)
                    nc.vector.tensor_copy(a3[:, c0:c1], act[:, 0:c1 - c0])
                    for g in range(8):
                        if (g0[g + 1] - 1) // 256 == grp:
                            gm = sp.tile([128, 1], F32, tag="gm")
                            nc.vector.tensor_reduce(
                                gm[:], a3[:, g0[g]:g0[g + 1]],
                                mybir.AxisListType.X, OP.max)
                            nc.vector.tensor_copy(gmpb[:, g:g + 1], gm[:])
                            ga = sp.tile([128, 1], F32, tag="ga")
                            nc.vector.tensor_reduce(
                                ga[:], a3[:, g0[g]:g0[g + 1]],
                                mybir.AxisListType.X, OP.add)
                            nc.vector.tensor_scalar_mul(ga[:], ga[:],
                                                        1.0 / cnt[g])
                            nc.vector.tensor_copy(gapb[:, g:g + 1], ga[:])

            # ---- layers: 3-stage pipeline (A, B lag-1, C lag-2) --------
            # piece starts: layers 1/2 use single-slot tier-split pieces
            # for the first 4 slots (overlap the AllGather tail)
            pstarts = []
            for l in range(3):
                d_ = {}
                if l == 0:
                    for p in range(0, NSLOT, PIECE):
                        d_[p] = min(p + PIECE, NSLOT)
                else:
                    for p in range(4):
                        d_[p] = p + 1
                    for p in range(4, NSLOT, PIECE):
                        d_[p] = min(p + PIECE, NSLOT)
                pstarts.append(d_)
            for l in range(3):
                nxh = 2 if l == 0 else 1
                zgrps = {}
                st = {}
                Mp = MTp = None
                pcur = 0
                for s in range(NSLOT + 2):
                    if 1 <= s <= NSLOT:
                        ps = s - 1
                        stage_b_den(l, ps, st[ps][2], st[ps][4])
                        st[ps] = st[ps][:5] + (stage_b_rdn(l, ps, st[ps][2]),)
                    if s >= 2:
                        ps = s - 2
                        S8 = stage_c_s8(l, ps, st[ps][2], st[ps][3])
                        if ps % GSLOT == 0:
                            zgrps[ps // GSLOT] = [
                                zgp.tile([128, H, GSLOT, TILE], BF16,
                                         tag=f"zg{xh}",
                                         name=f"zg{l}_{ps}_{xh}")
                                for xh in range(nxh)]
                    if s < NSLOT:
                        if s in pstarts[l]:
                            Mp, MTp = gather_piece(l, s, pstarts[l][s])
                            pcur = s
                        coff = int(slot_c0[s] - slot_c0[pcur])
                        zt, exf, exq = stage_a(l, s, Mp, MTp, coff)
                        st[s] = (Mp, coff, zt, exf, exq)
                    if 1 <= s <= NSLOT:
                        ps = s - 1
                        stage_b_rd(l, ps, st[ps][2], st[ps][5])
                    if s >= 2:
                        ps = s - 2
                        stage_c_agg(l, ps, st[ps][0], st[ps][1], st[ps][2],
                                    S8, zgrps[ps // GSLOT])
                        del st[ps]
                        if ps % GSLOT == GSLOT - 1:
                            grp = ps // GSLOT
                            evict_group(l, grp, zgrps.pop(grp))
                            if l < 2:
                                if grp == 1:
                                    emit_ag(l + 1, 0)
                                elif grp == 3:
                                    emit_ag(l + 1, 1)
                                elif grp == 4:
                                    emit_ag(l + 1, 2)
            # ---- readout ------------------------------------------------
            phg = pscr.tile([128, 8], F32, tag="scr")
            nc.tensor.matmul(phg[:], L0W[:, 0, :], gmpb[:], start=True,
                             stop=False)
            nc.tensor.matmul(phg[:], L0W[:, 1, :], gapb[:], start=False,
                             stop=True)
            hg = pp.tile([128, 8], BF16)
            nc.scalar.activation(hg[:], phg[:], AF.Relu, bias=L0B[:])
            pnw = pscr.tile([128, 8], F32, tag="scr")
            nc.tensor.matmul(pnw[:], LNW[:, 0, :], XR[:, 0, :], start=True,
                             stop=False)
            nc.tensor.matmul(pnw[:], LNW[:, 1, :], XR[:, 1, :], start=False,
                             stop=True)
            nw = pp.tile([128, 8], BF16)
            nc.scalar.activation(nw[:], pnw[:], AF.Relu, bias=LNB[:])
            pfin = pscr.tile([8, 1], F32, tag="scr")
            nc.tensor.matmul(pfin[:], hg[:], L1W[:, 0, :], start=True,
                             stop=False)
            nc.tensor.matmul(pfin[:], nw[:], L1W[:, 1, :], start=False,
                             stop=True)
            fin = pp.tile([8, 1], F32)
            nc.scalar.activation(fin[:], pfin[:], AF.Sigmoid, bias=L1B[:])
            nc.sync.dma_start(out_t[:], fin[:])
    nc.compile()
    return nc


def kernel(x, edge_index, batch,
           W1, as1, ad1, b1, g1, bb1, m1, v1,
           W2, as2, ad2, b2, g2, bb2, m2, v2,
           W3, as3, ad3, b3, g3, bb3, m3, v3,
           lnW, lnb, l0W, l0b, l1W, l1b):
    x = np.asarray(x, np.float32)
    edge_index = np.asarray(edge_index)
    batch = np.asarray(batch)
    Ws = [np.asarray(w, np.float64) for w in (W1, W2, W3)]
    ass = [np.asarray(a, np.float64) for a in (as1, as2, as3)]
    ads = [np.asarray(a, np.float64) for a in (ad1, ad2, ad3)]
    bs = [np.asarray(a, np.float32) for a in (b1, b2, b3)]
    gs = [np.asarray(a, np.float32) for a in (g1, g2, g3)]
    bbs = [np.asarray(a, np.float32) for a in (bb1, bb2, bb3)]
    ms = [np.asarray(a, np.float32) for a in (m1, m2, m3)]
    vs = [np.asarray(a, np.float32) for a in (v1, v2, v3)]

    src = np.concatenate([edge_index[0], np.arange(N)]).astype(np.int64)
    dst = np.concatenate([edge_index[1], np.arange(N)]).astype(np.int64)

    # ---- degree-balanced within-graph node->column permutation --------
    # Slot edge-loads should sit just under multiples of 128 to minimize
    # chunk padding (padded idxs cost real gather time).
    import bisect
    deg = np.bincount(dst, minlength=N).astype(np.int64)
    bnds0 = np.searchsorted(batch, np.arange(G + 1))
    Tk = np.array([deg[k * PER_CORE:(k + 1) * PER_CORE].sum()
                   for k in range(NCORES)])
    n4 = max(0, int(np.ceil((Tk.max() + 200 - NSLOT * 376) / 128.0)))
    n4 = min(n4, NSLOT)
    cap = np.full(NSLOT, 376, np.int64)
    if n4 > 0:
        for s in np.linspace(0, NSLOT - 1, n4).astype(int):
            cap[s] = 504
    col_of = np.zeros(N, np.int64)
    for k in range(NCORES):
        gidx = 8 * k
        pool_deg, pool_node = [], []
        rem = float(cap[0])
        for col in range(PER_CORE):
            gcol = k * PER_CORE + col
            s = col // TILE
            if col % TILE == 0 and col > 0:
                rem = float(cap[s])
            while gidx < 8 * (k + 1) and int(bnds0[gidx]) == gcol:
                lo, hi = int(bnds0[gidx]), int(bnds0[gidx + 1])
                order_g = np.argsort(deg[lo:hi], kind="stable")
                pool_node = list(lo + order_g)
                pool_deg = list(deg[lo:hi][order_g])
                gidx += 1
            m = min(TILE * (s + 1), PER_CORE) - col
            tgt = rem / max(m, 1)
            i = bisect.bisect_left(pool_deg, tgt)
            if i >= len(pool_deg):
                i = len(pool_deg) - 1
            elif i > 0 and (pool_deg[i] - tgt) > (tgt - pool_deg[i - 1]):
                i -= 1
            nsel = pool_node.pop(i)
            rem -= pool_deg.pop(i)
            col_of[nsel] = gcol
    node_at = np.zeros(N, np.int64)
    node_at[col_of] = np.arange(N)

    core = dst // PER_CORE
    dloc = col_of[dst] - core * PER_CORE
    slot = dloc // TILE
    din = dloc - slot * TILE

    # full-table row layout: AG-group-major, then core, then slot, then din
    grp_of_slot = np.zeros(NSLOT, np.int64)
    gb = np.zeros(3, np.int64)
    gs0 = np.zeros(3, np.int64)
    gn = np.zeros(3, np.int64)
    base = 0
    for gi_, (s0, s1) in enumerate(AG_GROUPS):
        grp_of_slot[s0:s1] = gi_
        gb[gi_], gs0[gi_], gn[gi_] = base, s0, s1 - s0
        base += (s1 - s0) * TILE * NCORES
    assert base == NFULL

    nodes_c = col_of                       # node -> global column
    k_n = nodes_c // PER_CORE
    dl_n = nodes_c - k_n * PER_CORE
    s_n = dl_n // TILE
    d_n = dl_n - s_n * TILE
    gi_n = grp_of_slot[s_n]
    frow = gb[gi_n] + k_n * gn[gi_n] * TILE + (s_n - gs0[gi_n]) * TILE + d_n
    assert frow.max() < NFULL and len(np.unique(frow)) == N

    # order edges by (core, slot, tier of src row)
    tier = (frow[src] >= 4096).astype(np.int64) + (frow[src] >= 8192)
    order = np.lexsort((tier, slot + 100 * core))
    srco = src[order]
    coreo = core[order]
    sloto = slot[order]
    dino = din[order]

    cnts = np.zeros((NCORES, NSLOT), np.int64)
    np.add.at(cnts, (coreo, sloto), 1)
    nct = np.maximum(
        np.ceil(cnts.max(axis=0) / 128).astype(np.int64), 1)
    nchunk = int(nct.sum())
    slot_c0 = np.concatenate([[0], np.cumsum(nct)]).astype(int)

    # st table position per chunk: phase j = s%4 rows; sequential per phase
    stpos = np.zeros(nchunk, np.int64)
    phase_pos = [0, 0, 0, 0]
    for s in range(NSLOT):
        j = s % 4
        for ci in range(nct[s]):
            stpos[slot_c0[s] + ci] = phase_pos[j]
            phase_pos[j] += 1
    CH4 = max(phase_pos)

    # per-edge chunk-local position (edges already (core,slot)-grouped)
    offs = np.zeros((NCORES, NSLOT), np.int64)
    flat = cnts.flatten()
    offs_flat = np.concatenate([[0], np.cumsum(flat)[:-1]])
    offs = offs_flat.reshape(NCORES, NSLOT)
    pos = np.arange(len(srco)) - offs[coreo, sloto]
    ch_of = slot_c0[sloto] + pos // 128
    e_of = pos % 128

    gsrc = np.zeros((NCORES, nchunk * 128), np.int16)
    gsrc[coreo, ch_of * 128 + e_of] = frow[srco]
    oh_np = np.zeros((NCORES, 128, nchunk, TILE), np.float32)
    oh_np[coreo, e_of, ch_of, dino] = 1.0
    st_np = np.zeros((NCORES, 128, CH4, 128), np.float32)
    st_np[coreo, 32 * (sloto % 4) + dino, stpos[ch_of], e_of] = 1.0

    gi = gsrc.reshape(NCORES, nchunk * 8, 16).transpose(0, 2, 1)
    gi = np.ascontiguousarray(np.tile(gi, (1, 8, 1)))

    # per-piece gather trim: skip trailing pad idxs of the piece's last slot
    PIECE = 2
    trim = {}
    for p in range(NSLOT):
        for last in (p, min(p + PIECE, NSLOT) - 1):
            nch_piece = int(slot_c0[last + 1] - slot_c0[p])
            lastfill = int(cnts[:, last].max() - (nct[last] - 1) * 128)
            lastfill = max(lastfill, 1)
            trim[(p, last + 1)] = (nch_piece - 1) * 128 + lastfill

    # chunk_lim: max row index any core's edges in this chunk may touch
    t0c = np.zeros((NCORES, NSLOT), np.int64)
    t1c = np.zeros((NCORES, NSLOT), np.int64)
    tiero = tier[order]
    np.add.at(t0c, (coreo, sloto), (tiero == 0).astype(np.int64))
    np.add.at(t1c, (coreo, sloto), (tiero <= 1).astype(np.int64))
    chunk_lim = np.full(nchunk, NFULL, np.int64)
    for s in range(NSLOT):
        for ci in range(int(nct[s])):
            e_end = (ci + 1) * 128
            if all(e_end <= t0c[k, s] for k in range(NCORES)):
                chunk_lim[slot_c0[s] + ci] = 4096
            elif all(e_end <= t1c[k, s] for k in range(NCORES)):
                chunk_lim[slot_c0[s] + ci] = 8192

    # layer-0 pregathered tiles (host-side static gather of input x)
    x8 = _f8(np.asarray(x, np.float32))                 # [N, 256]
    x8z = np.zeros((N + 1, 256), ml_dtypes.float8_e4m3)
    x8z[:N] = x8
    mx0_np = np.zeros((NCORES, 128, nchunk, 256), ml_dtypes.float8_e4m3)
    mx0_np[coreo, e_of, ch_of, :] = x8z[srco]
    mxt0_np = np.zeros((NCORES, 128, nchunk, 2, 128),
                       ml_dtypes.float8_e4m3)
    tmp = np.ascontiguousarray(x8z[srco].reshape(-1, 2, 128))
    mxt0_np[coreo, :, ch_of, :, e_of] = tmp.transpose(0, 2, 1)

    # weights
    wl = np.zeros((128, 4, H, 128), np.float64)
    wsd = np.zeros((128, 4, 16), np.float64)
    for li in range(3):
        W = Ws[li]
        ws = np.stack([W[:, h * C:(h + 1) * C] @ ass[li][h]
                       for h in range(H)], 1)
        wd = np.stack([W[:, h * C:(h + 1) * C] @ ads[li][h]
                       for h in range(H)], 1)
        if li == 0:
            for ki in range(2):
                for h in range(H):
                    wl[:, ki, h, :] = W[ki * 128:(ki + 1) * 128,
                                        h * C:(h + 1) * C]
                wsd[:, ki, 0:8] = ws[ki * 128:(ki + 1) * 128]
                wsd[:, ki, 8:16] = wd[ki * 128:(ki + 1) * 128]
        else:
            for h in range(H):
                wl[:, li + 1, h, :] = W[:, h * C:(h + 1) * C]
            wsd[:, li + 1, 0:8] = ws
            wsd[:, li + 1, 8:16] = wd

    wsdq = _f8(wsd[:, 0:2, 0:8])

    bn = np.zeros((128, 9), np.float32)
    for li in range(3):
        r = 1.0 / np.sqrt(vs[li] + EPS)
        bn[:, 3 * li + 0] = 8.0 * bs[li]
        bn[:, 3 * li + 1] = gs[li] * r / 8.0
        bn[:, 3 * li + 2] = bbs[li] - ms[li] * gs[li] * r

    bnds = np.searchsorted(batch, np.arange(G + 1))
    assert bnds[G] == N
    for k in range(1, NCORES):
        assert bnds[8 * k] == PER_CORE * k, "graphs must align to cores"
    g0 = [int(bnds[g]) for g in range(9)]
    cnt = [float(bnds[g + 1] - bnds[g]) for g in range(8)]
    for k in range(1, NCORES):
        for g in range(9):
            assert int(bnds[8 * k + g]) - PER_CORE * k == g0[g]
    root = bnds[:G]
    x64 = np.asarray(x, np.float64)
    xrT = x64[root].T.reshape(2, 128, G)

    l0w = _bf(np.asarray(l0W).reshape(2, 128, 128).transpose(1, 0, 2))
    lnw = _bf(np.asarray(lnW).reshape(2, 128, 128).transpose(1, 0, 2))
    l1w = _bf(np.asarray(l1W).reshape(2, 128, 1).transpose(1, 0, 2))

    nc = build_nc([int(v) for v in nct], [int(v) for v in stpos], g0, cnt,
                  trim, chunk_lim)

    in_maps = []
    for k in range(NCORES):
        xk = x64[node_at[k * PER_CORE:(k + 1) * PER_CORE]]   # [1250, 256]
        xTk = np.zeros((128, 2, RPAD), ml_dtypes.bfloat16)
        xTk[:, :, :PER_CORE] = _bf(np.ascontiguousarray(
            xk.T.reshape(2, 128, PER_CORE).transpose(1, 0, 2)))
        in_maps.append(dict(
            mx0=np.ascontiguousarray(mx0_np[k]),
            mxt0=np.ascontiguousarray(mxt0_np[k]),
            wsdq=wsdq, xT=xTk,
            oh=_f8(oh_np[k]), st=_bf(st_np[k]), gi=gi[k],
            wl=_bf(wl), wsd=_bf(wsd), bn=bn,
            ident=_bf(np.eye(128)),
            xrootT=_bf(np.ascontiguousarray(
                xrT[:, :, 8 * k:8 * k + 8].transpose(1, 0, 2))),
            l0w=l0w, lnw=lnw, l1w=l1w,
            l0b=np.asarray(l0b, np.float32).reshape(128, 1),
            lnb=np.asarray(lnb, np.float32).reshape(128, 1),
            l1b=np.broadcast_to(np.asarray(l1b, np.float32), (8, 1)).copy(),
        ))
    global LAST_RESULT, LAST_NC, LAST_INMAPS
    LAST_NC, LAST_INMAPS = nc, in_maps
    res = run_bass_kernel_spmd(nc, in_maps, core_ids=list(range(NCORES)))
    LAST_RESULT = res
    out = np.concatenate([res.results[k]["out"] for k in range(NCORES)], 0)
    return out.astype(np.float32)
